# revision 26
# baseline (speedup 1.0000x reference)
"""Trainium2 Bass kernel: mean over rows of ||A_row - B_row||_2.

Full inputs A, B: [2_000_000, 64] fp32. Data-parallel over 8 NeuronCores:
core c gets rows [c*250_000, (c+1)*250_000), padded with zero rows to
250_368 = 2 * 125_184 (pad rows give sqrt(0) = 0, contributing nothing;
the final piece is short, 768 columns, to keep the padding small).

Host side: A and B are quantized to fp8e4m3 (the rel-err budget is 2e-2;
fp8 input quantization costs ~7e-4) and laid out "transposed": SBUF/DRAM
partition p < 64 holds dim p of even rows, p >= 64 holds dim p-64 of odd
rows, so each column holds one row PAIR. A- and B-columns interleave, so
one DMA per chunk brings both. This cuts HBM traffic 4x vs fp32 (the
target_regime=memory roofline: ~32 MB/core at 360 GB/s ~= 90 us).

Device pipeline, per 1536-column piece (82 pieces/core, three square
paths interleaved so ACT, DVE and GPSIMD all sit just under the DMA
roofline):
  - ACT path: d = A - B via PE matmul (stationary [+I; -I] fp8,
    DoubleRow "halves" layout; moving = interleaved AB columns) -> d in
    PSUM fp32 -> ACT square -> fp8 sq. DoubleRow consumes the A and B
    k-tiles at 0.5 cyc per output column.
  - DVE/GP paths: DVE subtracts directly from the interleaved SBUF tile
    with stride-2 access patterns -> bf16 d (no PSUM; a DVE op may read
    at most one PSUM operand, so squaring from PSUM on DVE is illegal),
    then DVE (bf16) or GPSIMD (fp8) squares it.
  - Row sums via stationary-heavy matmuls: sq is the *stationary*
    operand (weight loads cost nothing), moving is a tiny ones matrix;
    out [128, 4] per 256 sq columns lands packed in a PSUM "rs" bank as
    one accumulation group (start=True only on the first write, which
    zeroes the whole bank). Rowsum matmuls trail their squares by PIPE
    pieces so the in-order PE queue never stalls on a pending square.
  - At each FLUSH_AT boundary one ACT sqrt(, accum_out=csum) pass
    reduces the rs bank to per-partition partial sums.
Host sums the 8 x 128 partials in f64 and divides by N.
"""

import sys

import numpy as np

for _p in ("/opt/trn_rl_repo",):
    if _p not in sys.path:
        sys.path.insert(0, _p)

import ml_dtypes

import concourse.bacc as bacc
import concourse.mybir as mybir
import concourse.tile as tile
from concourse.bass_utils import run_bass_kernel_spmd

NPFP8 = ml_dtypes.float8_e4m3
NPBF16 = ml_dtypes.bfloat16

N_ROWS = 2_000_000
D = 64
N_CORES = 8
ROWS_PER_CORE = N_ROWS // N_CORES  # 250_000

P = 128
PIECE = 1536                       # d columns per piece (3 PSUM banks)
NPIECE = 82                        # pieces per core (the last one short)
LAST_PIECE = 768                   # final piece is short to trim padding;
                                   # it runs on the DVE path (no PSUM need)
COLS = PIECE * (NPIECE - 1) + LAST_PIECE  # 125_184 column pairs
ROWS_PAD = 2 * COLS                # 250_368 rows per core (368 zero rows)
SUBW = 512                         # columns per subtract matmul
RS_SLOTS = 126                     # 256-col slots per rs bank (21 pieces)
NBANK = 5
# pieces after whose rowsums the rs bank is flushed (last bank kept small
# so the final sqrt only waits on the last three pieces)
FLUSH_AT = {20, 41, 62, 77, 81}

# Square-path schedule: per piece, one of ACT (square from PSUM -> fp8),
# DVE (copy PSUM->bf16 then DVE multiply), GP (copy then GPSIMD multiply).
# Ratios tuned so every engine sits just under the ~90 us DMA roofline.
N_ACT, N_DVE, N_GP = 45, 15, 22
PIPE = 16  # rowsum matmuls trail their square by this many pieces


def _path_schedule():
    """Interleave ACT/DVE/GP piece assignments evenly across the run, then
    force the final two pieces onto the DVE and ACT paths (their squares
    run in parallel on different engines) so the post-DMA drain is short."""
    sched = []
    cnt = {"ACT": 0, "DVE": 0, "GP": 0}
    tot = {"ACT": N_ACT, "DVE": N_DVE, "GP": N_GP}
    for k in range(NPIECE):
        best = max(tot, key=lambda s: tot[s] * (k + 1) - cnt[s] * NPIECE)
        sched.append(best)
        cnt[best] += 1
    want = {NPIECE - 2: "ACT", NPIECE - 1: "DVE"}
    for ti, w in want.items():
        if sched[ti] != w:
            for hi in range(ti - 1, -1, -1):
                if sched[hi] == w and hi not in want:
                    sched[ti], sched[hi] = sched[hi], sched[ti]
                    break
    return sched


_nc_cache = None
LAST_RESULTS = None  # BassKernelResults of the most recent run (for profiling)


def _build():
    f32 = mybir.dt.float32
    bf16 = mybir.dt.bfloat16
    fp8 = mybir.dt.float8e4
    MUL = mybir.AluOpType.mult
    DR = mybir.MatmulPerfMode.DoubleRow
    SQRT = mybir.ActivationFunctionType.Sqrt

    nc = bacc.Bacc(
        "TRN2", target_bir_lowering=False, debug=False, num_devices=N_CORES
    )
    XT = nc.dram_tensor("XT", [P, 2 * COLS], fp8, kind="ExternalInput").ap()
    WSUB = nc.dram_tensor("WSUB", [P, 256], fp8, kind="ExternalInput").ap()
    WONE8 = nc.dram_tensor("WONE8", [P, 8], fp8, kind="ExternalInput").ap()
    WONE16 = nc.dram_tensor("WONE16", [P, 2], bf16, kind="ExternalInput").ap()
    OUT = nc.dram_tensor("OUT", [P, NBANK], f32, kind="ExternalOutput").ap()

    sched = _path_schedule()

    with tile.TileContext(nc) as tc:
        with (
            tc.tile_pool(name="pw", bufs=1) as pw,
            tc.tile_pool(name="px", bufs=5) as px,
            tc.tile_pool(name="psq", bufs=PIPE + 2) as psq,
            tc.tile_pool(name="pdb", bufs=6) as pdb,
            tc.tile_pool(name="pd", bufs=2, space="PSUM") as pd,
            tc.tile_pool(name="prs", bufs=2, space="PSUM") as prs,
            tc.tile_pool(name="pacc", bufs=1) as pacc,
        ):
            wsub = pw.tile([P, 256], fp8)
            wone8 = pw.tile([P, 8], fp8)
            wone16 = pw.tile([P, 2], bf16)
            nc.scalar.dma_start(wsub[:], WSUB)
            nc.scalar.dma_start(wone8[:], WONE8)
            nc.scalar.dma_start(wone16[:], WONE16)
            wsub_ap = wsub[:].rearrange("p (two m) -> p two m", two=2)
            wone8_ap = wone8[:].rearrange("p (two c) -> p two c", two=2)

            scratch = pacc.tile([P, 4 * RS_SLOTS], f32)
            csum = pacc.tile([P, NBANK], f32)

            rsbank = None
            g = 0        # 256-col slots used in current bank
            bank_i = 0
            pending = []  # (sq_tile, path) awaiting rowsum emission

            def emit_rowsums(sq, path, ncols=PIECE):
                nonlocal rsbank, g, bank_i
                emit_rowsums.cur_piece += 1
                if rsbank is None:
                    rsbank = prs.tile([P, 512], f32, name="rs")
                    g = 0
                if path in ("ACT", "GP"):  # fp8 sq -> DoubleRow rowsums
                    for m in range(ncols // 256):
                        lhsT = sq[:, m * 256 : (m + 1) * 256].rearrange(
                            "p (two mm) -> p two mm", two=2
                        )
                        nc.tensor.matmul(
                            rsbank[:, 4 * g : 4 * g + 4],
                            lhsT,
                            wone8_ap,
                            start=(g == 0),
                            stop=False,
                            perf_mode=DR,
                            skip_group_check=True,
                        )
                        g += 1
                else:
                    for m in range(ncols // 256):
                        for h in range(2):
                            lhsT = sq[
                                :, m * 256 + h * 128 : m * 256 + h * 128 + 128
                            ]
                            nc.tensor.matmul(
                                rsbank[:, 4 * g + 2 * h : 4 * g + 2 * h + 2],
                                lhsT,
                                wone16[:],
                                start=(g == 0 and h == 0),
                                stop=False,
                                skip_group_check=True,
                            )
                        g += 1
                if emit_rowsums.cur_piece in FLUSH_AT and g > 0:
                    nc.scalar.activation(
                        scratch[:, : 4 * g],
                        rsbank[:, : 4 * g],
                        SQRT,
                        accum_out=csum[:, bank_i : bank_i + 1],
                    )
                    bank_i += 1
                    rsbank = None

            emit_rowsums.cur_piece = -1
            for k in range(NPIECE):
                ncols = LAST_PIECE if k == NPIECE - 1 else PIECE
                if k % 2 == 0:
                    xt = px.tile([P, 4 * PIECE], fp8)
                    dma_eng = nc.scalar if (k // 2) % 2 else nc.sync
                    pair_cols = (
                        PIECE + LAST_PIECE
                        if k + 1 == NPIECE - 1
                        else 2 * PIECE
                    )
                    with tc.high_priority():
                        dma_eng.dma_start(
                            xt[:, : 2 * pair_cols],
                            XT[:, k * 2 * PIECE : k * 2 * PIECE + 2 * pair_cols],
                        )
                    xoff = 0
                else:
                    xoff = 2 * PIECE

                path = sched[k]
                if path == "ACT":
                    # PE subtract -> PSUM, ACT square from PSUM -> fp8
                    dt_ = pd.tile([P, PIECE], f32)
                    for j in range(PIECE // SUBW):
                        rhs = xt[
                            :, xoff + j * 2 * SUBW : xoff + (j + 1) * 2 * SUBW
                        ].rearrange("p (r two) -> p two r", two=2)
                        nc.tensor.matmul(
                            dt_[:, j * SUBW : (j + 1) * SUBW],
                            wsub_ap,
                            rhs,
                            perf_mode=DR,
                            skip_group_check=True,
                        )
                    sq = psq.tile([P, PIECE], fp8, name="sq8")
                    nc.scalar.square(sq[:], dt_[:])
                else:
                    # DVE strided subtract straight from the interleaved tile
                    xa = xt[:, xoff : xoff + 2 * ncols].rearrange(
                        "p (r two) -> p two r", two=2
                    )
                    db = pdb.tile([P, PIECE], bf16, name="db")
                    nc.vector.tensor_tensor(
                        db[:, :ncols], xa[:, 0], xa[:, 1], mybir.AluOpType.subtract
                    )
                    if path == "GP":
                        sq = psq.tile([P, PIECE], fp8, name="sqg8")
                        with nc.allow_low_precision(reason="sq fp8"):
                            nc.gpsimd.tensor_tensor(
                                sq[:, :ncols], db[:, :ncols], db[:, :ncols], MUL
                            )
                    else:
                        sq = psq.tile([P, PIECE], bf16, name="sq16")
                        with nc.allow_low_precision(reason="sq bf16"):
                            nc.vector.tensor_tensor(
                                sq[:, :ncols], db[:, :ncols], db[:, :ncols], MUL
                            )
                pending.append((sq, path, ncols))

                if len(pending) > PIPE:
                    emit_rowsums(*pending.pop(0))

            for sq, path, ncols in pending:
                emit_rowsums(sq, path, ncols)
            assert rsbank is None and bank_i == NBANK

            nc.sync.dma_start(OUT, csum[:])
    nc.compile()
    return nc


def make_inputs(A, B):
    """[2M, 64] x2 -> per-core XT [8, 128, 2*COLS] fp8 plus weights."""
    A8 = np.zeros((N_CORES, ROWS_PAD, D), dtype=NPFP8)
    B8 = np.zeros((N_CORES, ROWS_PAD, D), dtype=NPFP8)
    A8[:, :ROWS_PER_CORE] = (
        np.asarray(A, dtype=np.float32).reshape(N_CORES, ROWS_PER_CORE, D)
    ).astype(NPFP8)
    B8[:, :ROWS_PER_CORE] = (
        np.asarray(B, dtype=np.float32).reshape(N_CORES, ROWS_PER_CORE, D)
    ).astype(NPFP8)
    # transpose to [core, 128, COLS]: partition = half*64 + dim, col = row pair
    XA = A8.reshape(N_CORES, COLS, 2, D).transpose(0, 2, 3, 1).reshape(
        N_CORES, P, COLS
    )
    XB = B8.reshape(N_CORES, COLS, 2, D).transpose(0, 2, 3, 1).reshape(
        N_CORES, P, COLS
    )
    XT = np.stack([XA, XB], axis=-1).reshape(N_CORES, P, 2 * COLS)

    wsub = np.zeros((P, 256), dtype=NPFP8)
    for p in range(P):
        wsub[p, p] = 1.0
        wsub[p, 128 + p] = -1.0
    wone8 = np.zeros((P, 8), dtype=NPFP8)
    for p in range(P):
        if p < 64:
            wone8[p, 0] = 1.0
            wone8[p, 4 + 2] = 1.0
        else:
            wone8[p, 1] = 1.0
            wone8[p, 4 + 3] = 1.0
    wone16 = np.zeros((P, 2), dtype=NPBF16)
    for p in range(P):
        wone16[p, 0 if p < 64 else 1] = 1.0
    return XT, wsub, wone8, wone16


def kernel(A, B):
    global _nc_cache, LAST_RESULTS
    XT, wsub, wone8, wone16 = make_inputs(A, B)
    if _nc_cache is None:
        _nc_cache = _build()
    nc = _nc_cache
    in_maps = [
        {"XT": XT[c], "WSUB": wsub, "WONE8": wone8, "WONE16": wone16}
        for c in range(N_CORES)
    ]
    res = run_bass_kernel_spmd(nc, in_maps, core_ids=list(range(N_CORES)))
    LAST_RESULTS = res
    total = 0.0
    for rmap in res.results:
        total += float(np.sum(rmap["OUT"].astype(np.float64)))
    # zero-padded rows contribute sqrt(0) = 0
    mean = total / N_ROWS
    return np.array(mean, dtype=np.float32)


# revision 28
# speedup vs baseline: 1.0105x; 1.0105x over previous
"""Trainium2 Bass kernel: mean over rows of ||A_row - B_row||_2.

Full inputs A, B: [2_000_000, 64] fp32. Data-parallel over 8 NeuronCores:
core c gets rows [c*250_000, (c+1)*250_000), padded with zero rows to
250_368 = 2 * 125_184 (pad rows give sqrt(0) = 0, contributing nothing;
the final piece is short, 768 columns, to keep the padding small).

Host side: A and B are quantized to fp8e4m3 (the rel-err budget is 2e-2;
fp8 input quantization costs ~7e-4) and laid out "transposed": SBUF/DRAM
partition p < 64 holds dim p of even rows, p >= 64 holds dim p-64 of odd
rows, so each column holds one row PAIR. A- and B-columns interleave, so
one DMA per chunk brings both. This cuts HBM traffic 4x vs fp32 (the
target_regime=memory roofline: ~32 MB/core at 360 GB/s ~= 90 us).

Device pipeline, per 1536-column piece (82 pieces/core, three square
paths interleaved so ACT, DVE and GPSIMD all sit just under the DMA
roofline):
  - ACT path: d = A - B via PE matmul (stationary [+I; -I] fp8,
    DoubleRow "halves" layout; moving = interleaved AB columns) -> d in
    PSUM fp32 -> ACT square -> fp8 sq. DoubleRow consumes the A and B
    k-tiles at 0.5 cyc per output column.
  - DVE/GP paths: DVE subtracts directly from the interleaved SBUF tile
    with stride-2 access patterns -> bf16 d (no PSUM; a DVE op may read
    at most one PSUM operand, so squaring from PSUM on DVE is illegal),
    then DVE (bf16) or GPSIMD (fp8) squares it.
  - Row sums via stationary-heavy matmuls: sq is the *stationary*
    operand (weight loads cost nothing), moving is a tiny ones matrix;
    out [128, 4] per 256 sq columns lands packed in a PSUM "rs" bank as
    one accumulation group (start=True only on the first write, which
    zeroes the whole bank). Rowsum matmuls trail their squares by PIPE
    pieces so the in-order PE queue never stalls on a pending square.
  - At each FLUSH_AT boundary one ACT sqrt(, accum_out=csum) pass
    reduces the rs bank to per-partition partial sums.
Host sums the 8 x 128 partials in f64 and divides by N.
"""

import sys

import numpy as np

for _p in ("/opt/trn_rl_repo",):
    if _p not in sys.path:
        sys.path.insert(0, _p)

import ml_dtypes

import concourse.bacc as bacc
import concourse.mybir as mybir
import concourse.tile as tile
from concourse.bass_utils import run_bass_kernel_spmd

NPFP8 = ml_dtypes.float8_e4m3
NPBF16 = ml_dtypes.bfloat16

N_ROWS = 2_000_000
D = 64
N_CORES = 8
ROWS_PER_CORE = N_ROWS // N_CORES  # 250_000

P = 128
PIECE = 1536                       # d columns per piece (3 PSUM banks)
NPIECE = 82                        # pieces per core (the last one short)
LAST_PIECE = 768                   # final piece is short to trim padding;
                                   # it runs on the DVE path (no PSUM need)
COLS = PIECE * (NPIECE - 1) + LAST_PIECE  # 125_184 column pairs
ROWS_PAD = 2 * COLS                # 250_368 rows per core (368 zero rows)
SUBW = 512                         # columns per subtract matmul
RS_SLOTS = 126                     # 256-col slots per rs bank (21 pieces)
NBANK = 5
# pieces after whose rowsums the rs bank is flushed (last bank kept small
# so the final sqrt only waits on the last three pieces)
FLUSH_AT = {18, 39, 59, 77, 81}

# Square-path schedule: per piece, one of ACT (square from PSUM -> fp8),
# DVE (copy PSUM->bf16 then DVE multiply), GP (copy then GPSIMD multiply).
# Ratios tuned so every engine sits just under the ~90 us DMA roofline.
N_ACT, N_DVE, N_GP = 45, 15, 22
PIPE = 16  # rowsum matmuls trail their square by this many pieces


def _path_schedule():
    """Interleave ACT/DVE/GP piece assignments evenly across the run, then
    force the final two pieces onto the DVE and ACT paths (their squares
    run in parallel on different engines) so the post-DMA drain is short."""
    sched = []
    cnt = {"ACT": 0, "DVE": 0, "GP": 0}
    tot = {"ACT": N_ACT, "DVE": N_DVE, "GP": N_GP}
    for k in range(NPIECE):
        best = max(tot, key=lambda s: tot[s] * (k + 1) - cnt[s] * NPIECE)
        sched.append(best)
        cnt[best] += 1
    want = {NPIECE - 2: "ACT", NPIECE - 1: "DVE"}
    for ti, w in want.items():
        if sched[ti] != w:
            for hi in range(ti - 1, -1, -1):
                if sched[hi] == w and hi not in want:
                    sched[ti], sched[hi] = sched[hi], sched[ti]
                    break
    return sched


_nc_cache = None
LAST_RESULTS = None  # BassKernelResults of the most recent run (for profiling)


def _build():
    f32 = mybir.dt.float32
    bf16 = mybir.dt.bfloat16
    fp8 = mybir.dt.float8e4
    MUL = mybir.AluOpType.mult
    DR = mybir.MatmulPerfMode.DoubleRow
    SQRT = mybir.ActivationFunctionType.Sqrt

    nc = bacc.Bacc(
        "TRN2", target_bir_lowering=False, debug=False, num_devices=N_CORES
    )
    XT = nc.dram_tensor("XT", [P, 2 * COLS], fp8, kind="ExternalInput").ap()
    WSUB = nc.dram_tensor("WSUB", [P, 256], fp8, kind="ExternalInput").ap()
    WONE8 = nc.dram_tensor("WONE8", [P, 8], fp8, kind="ExternalInput").ap()
    WONE16 = nc.dram_tensor("WONE16", [P, 2], bf16, kind="ExternalInput").ap()
    OUT = nc.dram_tensor("OUT", [P, NBANK], f32, kind="ExternalOutput").ap()

    sched = _path_schedule()

    with tile.TileContext(nc) as tc:
        with (
            tc.tile_pool(name="pw", bufs=1) as pw,
            tc.tile_pool(name="px", bufs=5) as px,
            tc.tile_pool(name="psq", bufs=PIPE + 2) as psq,
            tc.tile_pool(name="pdb", bufs=6) as pdb,
            tc.tile_pool(name="pd", bufs=2, space="PSUM") as pd,
            tc.tile_pool(name="prs", bufs=2, space="PSUM") as prs,
            tc.tile_pool(name="pacc", bufs=1) as pacc,
        ):
            wsub = pw.tile([P, 256], fp8)
            wone8 = pw.tile([P, 8], fp8)
            wone16 = pw.tile([P, 2], bf16)
            nc.scalar.dma_start(wsub[:], WSUB)
            nc.scalar.dma_start(wone8[:], WONE8)
            nc.scalar.dma_start(wone16[:], WONE16)
            wsub_ap = wsub[:].rearrange("p (two m) -> p two m", two=2)
            wone8_ap = wone8[:].rearrange("p (two c) -> p two c", two=2)

            scratch = pacc.tile([P, 4 * RS_SLOTS], f32)
            csum = pacc.tile([P, NBANK], f32)

            rsbank = None
            g = 0        # 256-col slots used in current bank
            bank_i = 0
            pending = []  # (sq_tile, path) awaiting rowsum emission

            def emit_rowsums(sq, path, ncols=PIECE):
                nonlocal rsbank, g, bank_i
                emit_rowsums.cur_piece += 1
                if rsbank is None:
                    rsbank = prs.tile([P, 512], f32, name="rs")
                    g = 0
                if path in ("ACT", "GP"):  # fp8 sq -> DoubleRow rowsums
                    for m in range(ncols // 256):
                        lhsT = sq[:, m * 256 : (m + 1) * 256].rearrange(
                            "p (two mm) -> p two mm", two=2
                        )
                        nc.tensor.matmul(
                            rsbank[:, 4 * g : 4 * g + 4],
                            lhsT,
                            wone8_ap,
                            start=(g == 0),
                            stop=False,
                            perf_mode=DR,
                            skip_group_check=True,
                        )
                        g += 1
                else:
                    for m in range(ncols // 256):
                        for h in range(2):
                            lhsT = sq[
                                :, m * 256 + h * 128 : m * 256 + h * 128 + 128
                            ]
                            nc.tensor.matmul(
                                rsbank[:, 4 * g + 2 * h : 4 * g + 2 * h + 2],
                                lhsT,
                                wone16[:],
                                start=(g == 0 and h == 0),
                                stop=False,
                                skip_group_check=True,
                            )
                        g += 1
                if emit_rowsums.cur_piece in FLUSH_AT and g > 0:
                    nc.scalar.activation(
                        scratch[:, : 4 * g],
                        rsbank[:, : 4 * g],
                        SQRT,
                        accum_out=csum[:, bank_i : bank_i + 1],
                    )
                    bank_i += 1
                    rsbank = None

            emit_rowsums.cur_piece = -1
            gp_tail = [k for k in range(NPIECE) if sched[k] == "GP"][-2:]
            for k in range(NPIECE):
                ncols = LAST_PIECE if k == NPIECE - 1 else PIECE
                if k % 2 == 0:
                    xt = px.tile([P, 4 * PIECE], fp8)
                    dma_eng = nc.scalar if (k // 2) % 2 else nc.sync
                    pair_cols = (
                        PIECE + LAST_PIECE
                        if k + 1 == NPIECE - 1
                        else 2 * PIECE
                    )
                    with tc.high_priority():
                        dma_eng.dma_start(
                            xt[:, : 2 * pair_cols],
                            XT[:, k * 2 * PIECE : k * 2 * PIECE + 2 * pair_cols],
                        )
                    xoff = 0
                else:
                    xoff = 2 * PIECE

                path = sched[k]
                if path == "ACT":
                    # PE subtract -> PSUM, ACT square from PSUM -> fp8
                    dt_ = pd.tile([P, PIECE], f32)
                    for j in range(PIECE // SUBW):
                        rhs = xt[
                            :, xoff + j * 2 * SUBW : xoff + (j + 1) * 2 * SUBW
                        ].rearrange("p (r two) -> p two r", two=2)
                        nc.tensor.matmul(
                            dt_[:, j * SUBW : (j + 1) * SUBW],
                            wsub_ap,
                            rhs,
                            perf_mode=DR,
                            skip_group_check=True,
                        )
                    sq = psq.tile([P, PIECE], fp8, name="sq8")
                    nc.scalar.square(sq[:], dt_[:])
                else:
                    # DVE strided subtract straight from the interleaved tile
                    xa = xt[:, xoff : xoff + 2 * ncols].rearrange(
                        "p (r two) -> p two r", two=2
                    )
                    db = pdb.tile([P, PIECE], bf16, name="db")
                    nc.vector.tensor_tensor(
                        db[:, :ncols], xa[:, 0], xa[:, 1], mybir.AluOpType.subtract
                    )
                    if path == "GP":
                        sq = psq.tile([P, PIECE], fp8, name="sqg8")
                        # the last GP squares sit on the drain critical path:
                        # split them GPSIMD/DVE so Pool finishes sooner
                        gpc = 1024 if k in gp_tail else ncols
                        with nc.allow_low_precision(reason="sq fp8"):
                            nc.gpsimd.tensor_tensor(
                                sq[:, :gpc], db[:, :gpc], db[:, :gpc], MUL
                            )
                            if gpc < ncols:
                                nc.vector.tensor_tensor(
                                    sq[:, gpc:ncols], db[:, gpc:ncols],
                                    db[:, gpc:ncols], MUL
                                )
                    else:
                        sq = psq.tile([P, PIECE], bf16, name="sq16")
                        with nc.allow_low_precision(reason="sq bf16"):
                            nc.vector.tensor_tensor(
                                sq[:, :ncols], db[:, :ncols], db[:, :ncols], MUL
                            )
                pending.append((sq, path, ncols))

                if len(pending) > PIPE:
                    emit_rowsums(*pending.pop(0))

            for sq, path, ncols in pending:
                emit_rowsums(sq, path, ncols)
            assert rsbank is None and bank_i == NBANK

            nc.sync.dma_start(OUT, csum[:])
    nc.compile()
    return nc


def make_inputs(A, B):
    """[2M, 64] x2 -> per-core XT [8, 128, 2*COLS] fp8 plus weights."""
    A8 = np.zeros((N_CORES, ROWS_PAD, D), dtype=NPFP8)
    B8 = np.zeros((N_CORES, ROWS_PAD, D), dtype=NPFP8)
    A8[:, :ROWS_PER_CORE] = (
        np.asarray(A, dtype=np.float32).reshape(N_CORES, ROWS_PER_CORE, D)
    ).astype(NPFP8)
    B8[:, :ROWS_PER_CORE] = (
        np.asarray(B, dtype=np.float32).reshape(N_CORES, ROWS_PER_CORE, D)
    ).astype(NPFP8)
    # transpose to [core, 128, COLS]: partition = half*64 + dim, col = row pair
    XA = A8.reshape(N_CORES, COLS, 2, D).transpose(0, 2, 3, 1).reshape(
        N_CORES, P, COLS
    )
    XB = B8.reshape(N_CORES, COLS, 2, D).transpose(0, 2, 3, 1).reshape(
        N_CORES, P, COLS
    )
    XT = np.stack([XA, XB], axis=-1).reshape(N_CORES, P, 2 * COLS)

    wsub = np.zeros((P, 256), dtype=NPFP8)
    for p in range(P):
        wsub[p, p] = 1.0
        wsub[p, 128 + p] = -1.0
    wone8 = np.zeros((P, 8), dtype=NPFP8)
    for p in range(P):
        if p < 64:
            wone8[p, 0] = 1.0
            wone8[p, 4 + 2] = 1.0
        else:
            wone8[p, 1] = 1.0
            wone8[p, 4 + 3] = 1.0
    wone16 = np.zeros((P, 2), dtype=NPBF16)
    for p in range(P):
        wone16[p, 0 if p < 64 else 1] = 1.0
    return XT, wsub, wone8, wone16


def kernel(A, B):
    global _nc_cache, LAST_RESULTS
    XT, wsub, wone8, wone16 = make_inputs(A, B)
    if _nc_cache is None:
        _nc_cache = _build()
    nc = _nc_cache
    in_maps = [
        {"XT": XT[c], "WSUB": wsub, "WONE8": wone8, "WONE16": wone16}
        for c in range(N_CORES)
    ]
    res = run_bass_kernel_spmd(nc, in_maps, core_ids=list(range(N_CORES)))
    LAST_RESULTS = res
    total = 0.0
    for rmap in res.results:
        total += float(np.sum(rmap["OUT"].astype(np.float64)))
    # zero-padded rows contribute sqrt(0) = 0
    mean = total / N_ROWS
    return np.array(mean, dtype=np.float32)


# revision 29
# speedup vs baseline: 1.0110x; 1.0005x over previous
"""Trainium2 Bass kernel: mean over rows of ||A_row - B_row||_2.

Full inputs A, B: [2_000_000, 64] fp32. Data-parallel over 8 NeuronCores:
core c gets rows [c*250_000, (c+1)*250_000), padded with zero rows to
250_368 = 2 * 125_184 (pad rows give sqrt(0) = 0, contributing nothing;
the final piece is short, 768 columns, to keep the padding small).

Host side: A and B are quantized to fp8e4m3 (the rel-err budget is 2e-2;
fp8 input quantization costs ~7e-4) and laid out "transposed": SBUF/DRAM
partition p < 64 holds dim p of even rows, p >= 64 holds dim p-64 of odd
rows, so each column holds one row PAIR. A- and B-columns interleave, so
one DMA per chunk brings both. This cuts HBM traffic 4x vs fp32 (the
target_regime=memory roofline: ~32 MB/core at 360 GB/s ~= 90 us).

Device pipeline, per 1536-column piece (82 pieces/core, three square
paths interleaved so ACT, DVE and GPSIMD all sit just under the DMA
roofline):
  - ACT path: d = A - B via PE matmul (stationary [+I; -I] fp8,
    DoubleRow "halves" layout; moving = interleaved AB columns) -> d in
    PSUM fp32 -> ACT square -> fp8 sq. DoubleRow consumes the A and B
    k-tiles at 0.5 cyc per output column.
  - DVE/GP paths: DVE subtracts directly from the interleaved SBUF tile
    with stride-2 access patterns -> bf16 d (no PSUM; a DVE op may read
    at most one PSUM operand, so squaring from PSUM on DVE is illegal),
    then DVE (bf16) or GPSIMD (fp8) squares it.
  - Row sums via stationary-heavy matmuls: sq is the *stationary*
    operand (weight loads cost nothing), moving is a tiny ones matrix;
    out [128, 4] per 256 sq columns lands packed in a PSUM "rs" bank as
    one accumulation group (start=True only on the first write, which
    zeroes the whole bank). Rowsum matmuls trail their squares by PIPE
    pieces so the in-order PE queue never stalls on a pending square.
  - At each FLUSH_AT boundary one ACT sqrt(, accum_out=csum) pass
    reduces the rs bank to per-partition partial sums.
Host sums the 8 x 128 partials in f64 and divides by N.
"""

import sys

import numpy as np

for _p in ("/opt/trn_rl_repo",):
    if _p not in sys.path:
        sys.path.insert(0, _p)

import ml_dtypes

import concourse.bacc as bacc
import concourse.mybir as mybir
import concourse.tile as tile
from concourse.bass_utils import run_bass_kernel_spmd

NPFP8 = ml_dtypes.float8_e4m3
NPBF16 = ml_dtypes.bfloat16

N_ROWS = 2_000_000
D = 64
N_CORES = 8
ROWS_PER_CORE = N_ROWS // N_CORES  # 250_000

P = 128
PIECE = 1536                       # d columns per piece (3 PSUM banks)
NPIECE = 82                        # pieces per core (the last one short)
LAST_PIECE = 768                   # final piece is short to trim padding;
                                   # it runs on the DVE path (no PSUM need)
COLS = PIECE * (NPIECE - 1) + LAST_PIECE  # 125_184 column pairs
ROWS_PAD = 2 * COLS                # 250_368 rows per core (368 zero rows)
SUBW = 512                         # columns per subtract matmul
RS_SLOTS = 126                     # 256-col slots per rs bank (21 pieces)
NBANK = 5
# pieces after whose rowsums the rs bank is flushed (last bank kept small
# so the final sqrt only waits on the last three pieces)
FLUSH_AT = {18, 39, 59, 77, 81}

# Square-path schedule: per piece, one of ACT (square from PSUM -> fp8),
# DVE (copy PSUM->bf16 then DVE multiply), GP (copy then GPSIMD multiply).
# Ratios tuned so every engine sits just under the ~90 us DMA roofline.
N_ACT, N_DVE, N_GP = 45, 15, 22
PIPE = 16  # rowsum matmuls trail their square by this many pieces


def _path_schedule():
    """Interleave ACT/DVE/GP piece assignments evenly across the run, then
    force the final two pieces onto the DVE and ACT paths (their squares
    run in parallel on different engines) so the post-DMA drain is short."""
    sched = []
    cnt = {"ACT": 0, "DVE": 0, "GP": 0}
    tot = {"ACT": N_ACT, "DVE": N_DVE, "GP": N_GP}
    for k in range(NPIECE):
        best = max(tot, key=lambda s: tot[s] * (k + 1) - cnt[s] * NPIECE)
        sched.append(best)
        cnt[best] += 1
    want = {NPIECE - 2: "ACT", NPIECE - 1: "DVE"}
    for ti, w in want.items():
        if sched[ti] != w:
            for hi in range(ti - 1, -1, -1):
                if sched[hi] == w and hi not in want:
                    sched[ti], sched[hi] = sched[hi], sched[ti]
                    break
    return sched


_nc_cache = None
LAST_RESULTS = None  # BassKernelResults of the most recent run (for profiling)


def _build():
    f32 = mybir.dt.float32
    bf16 = mybir.dt.bfloat16
    fp8 = mybir.dt.float8e4
    MUL = mybir.AluOpType.mult
    DR = mybir.MatmulPerfMode.DoubleRow
    SQRT = mybir.ActivationFunctionType.Sqrt

    nc = bacc.Bacc(
        "TRN2", target_bir_lowering=False, debug=False, num_devices=N_CORES
    )
    XT = nc.dram_tensor("XT", [P, 2 * COLS], fp8, kind="ExternalInput").ap()
    WSUB = nc.dram_tensor("WSUB", [P, 256], fp8, kind="ExternalInput").ap()
    WONE8 = nc.dram_tensor("WONE8", [P, 8], fp8, kind="ExternalInput").ap()
    WONE16 = nc.dram_tensor("WONE16", [P, 2], bf16, kind="ExternalInput").ap()
    OUT = nc.dram_tensor("OUT", [P, NBANK], f32, kind="ExternalOutput").ap()

    sched = _path_schedule()

    with tile.TileContext(nc) as tc:
        with (
            tc.tile_pool(name="pw", bufs=1) as pw,
            tc.tile_pool(name="px", bufs=5) as px,
            tc.tile_pool(name="psq", bufs=PIPE + 2) as psq,
            tc.tile_pool(name="pdb", bufs=6) as pdb,
            tc.tile_pool(name="pd", bufs=2, space="PSUM") as pd,
            tc.tile_pool(name="prs", bufs=2, space="PSUM") as prs,
            tc.tile_pool(name="pacc", bufs=1) as pacc,
        ):
            wsub = pw.tile([P, 256], fp8)
            wone8 = pw.tile([P, 8], fp8)
            wone16 = pw.tile([P, 2], bf16)
            nc.scalar.dma_start(wsub[:], WSUB)
            nc.scalar.dma_start(wone8[:], WONE8)
            nc.scalar.dma_start(wone16[:], WONE16)
            wsub_ap = wsub[:].rearrange("p (two m) -> p two m", two=2)
            wone8_ap = wone8[:].rearrange("p (two c) -> p two c", two=2)

            scratch = pacc.tile([P, 4 * RS_SLOTS], f32)
            csum = pacc.tile([P, NBANK], f32)

            rsbank = None
            g = 0        # 256-col slots used in current bank
            bank_i = 0
            pending = []  # (sq_tile, path) awaiting rowsum emission

            def emit_rowsums(sq, path, ncols=PIECE):
                nonlocal rsbank, g, bank_i
                emit_rowsums.cur_piece += 1
                if rsbank is None:
                    rsbank = prs.tile([P, 512], f32, name="rs")
                    g = 0
                if path in ("ACT", "GP"):  # fp8 sq -> DoubleRow rowsums
                    for m in range(ncols // 256):
                        lhsT = sq[:, m * 256 : (m + 1) * 256].rearrange(
                            "p (two mm) -> p two mm", two=2
                        )
                        nc.tensor.matmul(
                            rsbank[:, 4 * g : 4 * g + 4],
                            lhsT,
                            wone8_ap,
                            start=(g == 0),
                            stop=False,
                            perf_mode=DR,
                            skip_group_check=True,
                        )
                        g += 1
                else:
                    for m in range(ncols // 256):
                        for h in range(2):
                            lhsT = sq[
                                :, m * 256 + h * 128 : m * 256 + h * 128 + 128
                            ]
                            nc.tensor.matmul(
                                rsbank[:, 4 * g + 2 * h : 4 * g + 2 * h + 2],
                                lhsT,
                                wone16[:],
                                start=(g == 0 and h == 0),
                                stop=False,
                                skip_group_check=True,
                            )
                        g += 1
                if emit_rowsums.cur_piece in FLUSH_AT and g > 0:
                    nc.scalar.activation(
                        scratch[:, : 4 * g],
                        rsbank[:, : 4 * g],
                        SQRT,
                        accum_out=csum[:, bank_i : bank_i + 1],
                    )
                    bank_i += 1
                    rsbank = None

            emit_rowsums.cur_piece = -1
            gp_tail = [k for k in range(NPIECE) if sched[k] == "GP"][-2:]
            for k in range(NPIECE):
                ncols = LAST_PIECE if k == NPIECE - 1 else PIECE
                if k >= NPIECE - 2:
                    # final two pieces get their own DMAs so the
                    # second-to-last piece's data lands sooner
                    xt = px.tile([P, 4 * PIECE], fp8)
                    dma_eng = nc.scalar if k % 2 else nc.sync
                    with tc.high_priority():
                        dma_eng.dma_start(
                            xt[:, : 2 * ncols],
                            XT[:, k * 2 * PIECE : k * 2 * PIECE + 2 * ncols],
                        )
                    xoff = 0
                elif k % 2 == 0:
                    xt = px.tile([P, 4 * PIECE], fp8)
                    dma_eng = nc.scalar if (k // 2) % 2 else nc.sync
                    pair_cols = (
                        PIECE + LAST_PIECE
                        if k + 1 == NPIECE - 1
                        else 2 * PIECE
                    )
                    with tc.high_priority():
                        dma_eng.dma_start(
                            xt[:, : 2 * pair_cols],
                            XT[:, k * 2 * PIECE : k * 2 * PIECE + 2 * pair_cols],
                        )
                    xoff = 0
                else:
                    xoff = 2 * PIECE

                path = sched[k]
                if path == "ACT":
                    # PE subtract -> PSUM, ACT square from PSUM -> fp8
                    dt_ = pd.tile([P, PIECE], f32)
                    for j in range(PIECE // SUBW):
                        rhs = xt[
                            :, xoff + j * 2 * SUBW : xoff + (j + 1) * 2 * SUBW
                        ].rearrange("p (r two) -> p two r", two=2)
                        nc.tensor.matmul(
                            dt_[:, j * SUBW : (j + 1) * SUBW],
                            wsub_ap,
                            rhs,
                            perf_mode=DR,
                            skip_group_check=True,
                        )
                    sq = psq.tile([P, PIECE], fp8, name="sq8")
                    nc.scalar.square(sq[:], dt_[:])
                else:
                    # DVE strided subtract straight from the interleaved tile
                    xa = xt[:, xoff : xoff + 2 * ncols].rearrange(
                        "p (r two) -> p two r", two=2
                    )
                    db = pdb.tile([P, PIECE], bf16, name="db")
                    nc.vector.tensor_tensor(
                        db[:, :ncols], xa[:, 0], xa[:, 1], mybir.AluOpType.subtract
                    )
                    if path == "GP":
                        sq = psq.tile([P, PIECE], fp8, name="sqg8")
                        # the last GP squares sit on the drain critical path:
                        # split them GPSIMD/DVE so Pool finishes sooner
                        gpc = 1024 if k in gp_tail else ncols
                        with nc.allow_low_precision(reason="sq fp8"):
                            nc.gpsimd.tensor_tensor(
                                sq[:, :gpc], db[:, :gpc], db[:, :gpc], MUL
                            )
                            if gpc < ncols:
                                nc.vector.tensor_tensor(
                                    sq[:, gpc:ncols], db[:, gpc:ncols],
                                    db[:, gpc:ncols], MUL
                                )
                    else:
                        sq = psq.tile([P, PIECE], bf16, name="sq16")
                        with nc.allow_low_precision(reason="sq bf16"):
                            nc.vector.tensor_tensor(
                                sq[:, :ncols], db[:, :ncols], db[:, :ncols], MUL
                            )
                pending.append((sq, path, ncols))

                if len(pending) > PIPE:
                    emit_rowsums(*pending.pop(0))

            for sq, path, ncols in pending:
                emit_rowsums(sq, path, ncols)
            assert rsbank is None and bank_i == NBANK

            nc.sync.dma_start(OUT, csum[:])
    nc.compile()
    return nc


def make_inputs(A, B):
    """[2M, 64] x2 -> per-core XT [8, 128, 2*COLS] fp8 plus weights."""
    A8 = np.zeros((N_CORES, ROWS_PAD, D), dtype=NPFP8)
    B8 = np.zeros((N_CORES, ROWS_PAD, D), dtype=NPFP8)
    A8[:, :ROWS_PER_CORE] = (
        np.asarray(A, dtype=np.float32).reshape(N_CORES, ROWS_PER_CORE, D)
    ).astype(NPFP8)
    B8[:, :ROWS_PER_CORE] = (
        np.asarray(B, dtype=np.float32).reshape(N_CORES, ROWS_PER_CORE, D)
    ).astype(NPFP8)
    # transpose to [core, 128, COLS]: partition = half*64 + dim, col = row pair
    XA = A8.reshape(N_CORES, COLS, 2, D).transpose(0, 2, 3, 1).reshape(
        N_CORES, P, COLS
    )
    XB = B8.reshape(N_CORES, COLS, 2, D).transpose(0, 2, 3, 1).reshape(
        N_CORES, P, COLS
    )
    XT = np.stack([XA, XB], axis=-1).reshape(N_CORES, P, 2 * COLS)

    wsub = np.zeros((P, 256), dtype=NPFP8)
    for p in range(P):
        wsub[p, p] = 1.0
        wsub[p, 128 + p] = -1.0
    wone8 = np.zeros((P, 8), dtype=NPFP8)
    for p in range(P):
        if p < 64:
            wone8[p, 0] = 1.0
            wone8[p, 4 + 2] = 1.0
        else:
            wone8[p, 1] = 1.0
            wone8[p, 4 + 3] = 1.0
    wone16 = np.zeros((P, 2), dtype=NPBF16)
    for p in range(P):
        wone16[p, 0 if p < 64 else 1] = 1.0
    return XT, wsub, wone8, wone16


def kernel(A, B):
    global _nc_cache, LAST_RESULTS
    XT, wsub, wone8, wone16 = make_inputs(A, B)
    if _nc_cache is None:
        _nc_cache = _build()
    nc = _nc_cache
    in_maps = [
        {"XT": XT[c], "WSUB": wsub, "WONE8": wone8, "WONE16": wone16}
        for c in range(N_CORES)
    ]
    res = run_bass_kernel_spmd(nc, in_maps, core_ids=list(range(N_CORES)))
    LAST_RESULTS = res
    total = 0.0
    for rmap in res.results:
        total += float(np.sum(rmap["OUT"].astype(np.float64)))
    # zero-padded rows contribute sqrt(0) = 0
    mean = total / N_ROWS
    return np.array(mean, dtype=np.float32)
